# revision 9
# baseline (speedup 1.0000x reference)
"""Trainium2 Bass kernel for nn_Basicgate_patch_iv_multivoxel.

Math (see reference): two last-write-wins point scatters into a dense
[H,W] grid, a chain of 1x1 convs, a 3x3 conv to a 1-channel attention
map, sigmoid gate applied to x_rgb0.

All convs are linear, so they are folded down onto the sparse points:
  z = conv3x3(fused, Wsp) + bsp, where
  fused = gated + W2@W0@P0 + W2@P1 + (W2@b0 + b2),  gated = W3@x + b3.
Per winning point, its 9-tap stamp values E_t = (Wsp_t . M_l phi) are
gathered per-cell from an on-chip table via the winner map; the 3x3
spatial combine is done with one-hot PE matmuls over a 16-channel
"stamp" tile (9 taps + the dense gated map in channel 15); constants
and border effects live in a host-precomputed constant map.

Sharding: 8 cores = batch (2) x W-slabs (4, 160 cols + 1 halo col).
Host work is ONLY sharding/marshalling: bucketing points by their
target cell's (group, mod-16) bucket (order-preserving), transposes,
and weight folding. Winner resolution (last-write-wins) happens on
device via gpsimd local_scatter; per-cell stamp rows are fetched with
gpsimd ap_gather; everything else is PE/DVE/ACT/DMA.
"""
import sys

if '/opt/trn_rl_repo' not in sys.path:
    sys.path.insert(0, '/opt/trn_rl_repo')

import numpy as np

H, W, B = 192, 640, 2
NSLAB = 4
SW = 160          # interior cols per slab
SWH = 162         # incl. halo cols
NG = 8            # row groups per core
GR = 24           # output rows per group
WR = 26           # window rows per group (GR + 2 halo)
WIN = WR * SWH    # 4212 window cells
NE = 264          # winner-grid slots per partition (ceil(WIN/16))
NWIN = NE * 16    # 4224
NCELL = H * SWH   # 31104
CAPQ = 16
KEEP_LAST = 4     # duplicates kept per cell (winner is always among them)

_BUILD_CACHE = {}


def _fold_weights(W_red0, b_red0, W_red2, b_red2, W_red3, b_red3, W_sp, b_sp):
    Wsp9 = W_sp[0].reshape(67, 9).T.astype(np.float64)        # [9,67], t=3*dy+dx
    M0 = (Wsp9 @ W_red2.astype(np.float64) @ W_red0.astype(np.float64))  # [9,35]
    M1 = (Wsp9 @ W_red2.astype(np.float64))                   # [9,67]
    wsum = Wsp9.sum(axis=1)                                   # [9]
    Ky = W_red2.astype(np.float64) @ b_red0.astype(np.float64) + b_red2  # [67]
    cadd = Wsp9 @ Ky + float(b_red3[0]) * wsum                # [9]

    M0p = np.zeros((16, 35), np.float32); M0p[:9] = M0.astype(np.float32)
    M1p = np.zeros((16, 67), np.float32); M1p[:9] = M1.astype(np.float32)
    a0 = np.ascontiguousarray(M0p[np.arange(128) % 16].T)     # [35,128]
    a1 = np.ascontiguousarray(M1p[np.arange(128) % 16].T)     # [67,128]

    lc = np.zeros((128, 80), np.float32)
    for t in range(9):
        for m in range(8):
            lc[16 * m + t, 8 * t + m] += 1.0
            lc[16 * m + 15, 8 * t + m] += np.float32(wsum[t])
    return a0, a1, lc, cadd, float(b_sp[0])


def _const_map(s, cadd, bsp):
    # C[y, xa] = bsp + sum over valid taps of cadd[t]
    x0 = s * SW
    y = np.arange(H)[:, None]
    xa = x0 + np.arange(SWH)[None, :] - 1
    C = np.full((H, SWH), bsp, np.float64)
    for t in range(9):
        dy, dx = t // 3, t % 3
        valid = (y + dy - 1 >= 0) & (y + dy - 1 <= H - 1) & \
                (xa + dx - 1 >= 0) & (xa + dx - 1 <= W - 1)
        C += np.where(valid, cadd[t], 0.0)
    # to [8, 3888]: row g, beta = (R-24g)*162 + c,  R = 24g + beta//162
    cm = C.reshape(NG, GR * SWH).astype(np.float32)
    return np.ascontiguousarray(cm)


def _bucketize(coor, featvox, s):
    """Order-preserving bucket of points into 128 (group, j%16) buckets."""
    u = np.clip(coor[:, 0], 0.0, 1.0)
    v = np.clip(coor[:, 1], 0.0, 1.0)
    col = (u * W).astype(np.int32)
    row = (v * H).astype(np.int32)
    x0 = s * SW
    keep = (row <= H - 1) & (col <= W - 1) & (col >= x0 - 1) & (col <= x0 + SW)
    idxs = np.nonzero(keep)[0]
    row_k, col_k = row[idxs], col[idxs]
    ps, slots, pts = [], [], []
    for cand in (0, 1):
        g = (row_k + 1) // GR - cand
        ok = (g >= 0) & (g <= NG - 1) & (row_k >= GR * g - 1) & (row_k <= GR * g + GR)
        gg, rr, cc, pp = g[ok], row_k[ok], col_k[ok], idxs[ok]
        j = (rr - (GR * gg - 1)) * SWH + (cc - x0 + 1)
        ps.append(16 * gg + (j % 16))
        slots.append(j // 16)
        pts.append(pp)
    p_all = np.concatenate(ps)
    slot_all = np.concatenate(slots)
    pt_all = np.concatenate(pts)
    # Drop provably-dominated duplicates: for each cell keep only the last
    # KEEP_LAST points (winner = max original index is always among them, so
    # device-side last-write-wins resolution is unchanged). Purely an index-
    # level capacity bound for the shard buckets; no feature math on host.
    cellkey = p_all.astype(np.int64) * NE + slot_all
    order0 = np.lexsort((pt_all, cellkey))
    ck_s = cellkey[order0]
    seg_start = np.r_[0, np.nonzero(np.diff(ck_s))[0] + 1]
    seg_id = np.cumsum(np.r_[0, np.diff(ck_s) != 0])
    pos_in_seg = np.arange(len(ck_s)) - seg_start[seg_id]
    seg_len = np.r_[seg_start[1:], len(ck_s)] - seg_start
    keep = pos_in_seg >= seg_len[seg_id] - KEEP_LAST
    kept = order0[keep]
    p_all, slot_all, pt_all = p_all[kept], slot_all[kept], pt_all[kept]
    order = np.lexsort((pt_all, p_all))   # stable by (bucket, original idx)
    p_s, slot_s, pt_s = p_all[order], slot_all[order], pt_all[order]
    counts = np.bincount(p_s, minlength=128)
    cap = int(max(CAPQ, -(-(counts.max() + 1) // CAPQ) * CAPQ))
    offs = np.zeros(128, np.int64)
    offs[1:] = np.cumsum(counts)[:-1]
    rank = np.arange(len(p_s)) - offs[p_s]
    ls_idx = np.full((128, cap), -1, np.int16)
    ls_idx[p_s, rank] = slot_s.astype(np.int16)
    CF = featvox.shape[1]
    fvT = np.zeros((CF, 128 * cap), np.float32)
    fvT[:, p_s * cap + rank] = featvox[pt_s].T
    return ls_idx, fvT, cap


def _build(cap0, cap1):
    key = (cap0, cap1)
    if key in _BUILD_CACHE:
        return _BUILD_CACHE[key]
    import concourse.bass as bass
    import concourse.tile as tile
    from concourse import bacc, mybir, library_config
    dt = mybir.dt
    NT0, NT1 = 128 * cap0, 128 * cap1
    TW0, TW1 = NT0 + 4, NT1 + 4

    nc = bacc.Bacc("TRN2", target_bir_lowering=False, debug=False, num_devices=8)
    f32, i16, i32 = dt.float32, dt.int16, dt.int32

    def din(name, shape, dtype=f32):
        return nc.dram_tensor(name, shape, dtype, kind="ExternalInput").ap()

    xin_d = din("xin", [128, 243 * 64])
    w3b_d = din("w3b", [128, 64])
    fv_d = [din("fv0", [35, NT0]), din("fv1", [67, NT1])]
    lsx_d = [din("lsx0", [128, cap0], i16), din("lsx1", [128, cap1], i16)]
    a_d = [din("a0", [35, 128]), din("a1", [67, 128])]
    lc_d = din("lc", [128, 80])
    cm_d = din("cm", [8, GR * SWH])
    yout_d = nc.dram_tensor("yout", [128, 243 * 64], f32, kind="ExternalOutput").ap()
    gdram = nc.dram_tensor("gdram", [128, 243], f32).ap()
    adram = nc.dram_tensor("adram", [8, GR * SWH], f32).ap()

    caps = (cap0, cap1)
    tws = (TW0, TW1)
    nts = (NT0, NT1)
    OFFS = [dy * SWH + dx - 1 for dy in range(3) for dx in range(3)]
    NHALF = NWIN // 2          # 2112
    CHW = 486                  # combine chunk width (8 * 486 = 3888)

    with tile.TileContext(nc) as tc:
        with tc.tile_pool(name="per", bufs=1) as per, \
             tc.tile_pool(name="fvp", bufs=3) as fvp, \
             tc.tile_pool(name="xp", bufs=2) as xp, \
             tc.tile_pool(name="sm", bufs=2) as sm, \
             tc.tile_pool(name="pe", bufs=2, space="PSUM") as pe, \
             tc.tile_pool(name="pc", bufs=2, space="PSUM") as pc:

            # ---- winner resolution (gpsimd local_scatter) ----
            nc.gpsimd.load_library(library_config.local_scatter)
            wg = []
            for l in range(2):
                lsx_t = per.tile([128, caps[l]], i16, tag=f"lsx{l}")
                nc.sync.dma_start(lsx_t[:], lsx_d[l][:])
                dat_t = per.tile([128, caps[l]], i16, tag=f"dat{l}")
                nc.gpsimd.iota(dat_t[:], pattern=[[1, caps[l]]], base=1,
                               channel_multiplier=0)
                wg_t = per.tile([128, NE], i16, tag=f"wg{l}")
                nc.gpsimd.local_scatter(wg_t[:], dat_t[:], lsx_t[:],
                                        channels=128, num_elems=NE,
                                        num_idxs=caps[l])
                wg.append(wg_t)

            # ---- E tables via PE ----
            etab = []
            for l in range(2):
                et = per.tile([128, tws[l]], f32, tag=f"etab{l}")
                CF = 35 if l == 0 else 67
                at = per.tile([CF, 128], f32, tag=f"arep{l}")
                nc.sync.dma_start(at[:], a_d[l][:])
                nchunks = nts[l] // 512
                for c in range(nchunks):
                    rhs = fvp.tile([67, 512], f32, tag="fvchunk")
                    nc.sync.dma_start(rhs[:CF, :],
                                      fv_d[l][:, c * 512:(c + 1) * 512])
                    ps_t = pe.tile([128, 512], f32, tag="eps")
                    nc.tensor.matmul(out=ps_t[:], lhsT=at[:], rhs=rhs[:CF, :],
                                     start=True, stop=True)
                    nc.vector.tensor_copy(et[:, c * 512:(c + 1) * 512], ps_t[:])
                nc.vector.memset(et[:, nts[l]:tws[l]], 0.0)
                etab.append(et)

            # ---- winner -> table index (DVE) ----
            ix = []
            for l in range(2):
                pci = per.tile([128, 1], i32, tag=f"pci{l}")
                nc.gpsimd.iota(pci[:], pattern=[[1, 1]], base=0,
                               channel_multiplier=caps[l])
                pcf = per.tile([128, 1], f32, tag=f"pcf{l}")
                nc.vector.tensor_copy(pcf[:], pci[:])
                wf = sm.tile([128, NE], f32, tag="wf")
                nc.vector.tensor_copy(wf[:], wg[l][:])
                eq = sm.tile([128, NE], f32, tag="eq")
                nc.vector.tensor_scalar(eq[:], wf[:], 0.0, None,
                                        op0=mybir.AluOpType.is_equal)
                nc.vector.tensor_scalar(wf[:], wf[:], -1.0, None,
                                        op0=mybir.AluOpType.add)
                nc.vector.tensor_tensor(wf[:], wf[:],
                                        pcf[:].to_broadcast([128, NE]),
                                        op=mybir.AluOpType.add)
                tmp = sm.tile([128, NE], f32, tag="ixtmp")
                nc.vector.tensor_tensor(tmp[:], eq[:], wf[:],
                                        op=mybir.AluOpType.mult)
                nc.vector.tensor_tensor(wf[:], wf[:], tmp[:],
                                        op=mybir.AluOpType.subtract)
                nc.vector.tensor_scalar(eq[:], eq[:], float(nts[l]), None,
                                        op0=mybir.AluOpType.mult)
                nc.vector.tensor_tensor(wf[:], wf[:], eq[:],
                                        op=mybir.AluOpType.add)
                ix_t = per.tile([128, NE], i16, tag=f"ix{l}")
                nc.vector.tensor_copy(ix_t[:], wf[:])
                ix.append(ix_t)

            # ---- gated map: stream x, 1x1 reduce (DVE) ----
            w3b_t = per.tile([128, 64], f32, tag="w3b")
            nc.sync.dma_start(w3b_t[:], w3b_d[:])
            gcp = per.tile([128, 243], f32, tag="gcp")
            CCH = (41, 41, 41, 41, 41, 38)
            coff = 0
            for ck in CCH:
                xc = xp.tile([128, 41 * 64], f32, tag="xa")
                nc.sync.dma_start(xc[:, :ck * 64],
                                  xin_d[:, coff * 64:(coff + ck) * 64])
                xv = xc[:, :ck * 64].rearrange("p (n c) -> p n c", c=64)
                nc.vector.tensor_tensor(
                    xv, xv,
                    w3b_t[:].rearrange("p (o c) -> p o c", o=1)
                           .to_broadcast([128, ck, 64]),
                    op=mybir.AluOpType.mult)
                nc.vector.tensor_reduce(gcp[:, coff:coff + ck], xv,
                                        axis=mybir.AxisListType.X,
                                        op=mybir.AluOpType.add)
                coff += ck
            nc.sync.dma_start(gdram[:], gcp[:])

            # ---- stamp tile S9 (gathers) + gated windows ----
            s9 = per.tile([128, NWIN + 4], f32, tag="s9")
            nc.vector.memset(s9[:, 0:2], 0.0)
            nc.vector.memset(s9[:, NWIN + 2:], 0.0)
            s9b = per.tile([128, NHALF], f32, tag="s9b")
            nc.gpsimd.load_library(library_config.ap_gather)
            for h in range(2):
                o = h * NHALF
                nc.gpsimd.ap_gather(s9[:, 2 + o:2 + o + NHALF], etab[0][:],
                                    ix[0][:, o // 16:(o + NHALF) // 16],
                                    channels=128, num_elems=tws[0], d=1,
                                    num_idxs=NHALF)
                nc.gpsimd.ap_gather(s9b[:], etab[1][:],
                                    ix[1][:, o // 16:(o + NHALF) // 16],
                                    channels=128, num_elems=tws[1], d=1,
                                    num_idxs=NHALF)
                nc.vector.tensor_tensor(s9[:, 2 + o:2 + o + NHALF],
                                        s9[:, 2 + o:2 + o + NHALF], s9b[:],
                                        op=mybir.AluOpType.add)
            gfl = gdram.rearrange("p n -> (p n)")
            for g in range(NG):
                lo = (GR * g - 1) * SWH
                hi = (GR * g + GR + 1) * SWH
                lo_c, hi_c = max(lo, 0), min(hi, NCELL)
                off = 2 + (lo_c - lo)
                nc.sync.dma_start(s9[16 * g + 15:16 * g + 16,
                                     off:off + (hi_c - lo_c)],
                                  gfl[lo_c:hi_c])

            # ---- combine: one-hot matmuls + cmap + sigmoid ----
            lc_t = per.tile([128, 80], f32, tag="lct")
            nc.sync.dma_start(lc_t[:], lc_d[:])
            for ci in range(8):
                b0 = ci * CHW
                cm_t = sm.tile([8, CHW], f32, tag="cmt")
                nc.sync.dma_start(cm_t[:], cm_d[:, b0:b0 + CHW])
                psc = pc.tile([8, CHW], f32, tag="cps")
                for t in range(9):
                    nc.tensor.matmul(out=psc[:],
                                     lhsT=lc_t[:, 8 * t:8 * t + 8],
                                     rhs=s9[:, 2 + b0 + OFFS[t]:
                                            2 + b0 + OFFS[t] + CHW],
                                     start=(t == 0), stop=(t == 8))
                az = sm.tile([8, CHW], f32, tag="az")
                nc.vector.tensor_tensor(az[:], psc[:], cm_t[:],
                                        op=mybir.AluOpType.add)
                at_t = sm.tile([8, CHW], f32, tag="att")
                nc.scalar.activation(at_t[:], az[:],
                                     mybir.ActivationFunctionType.Sigmoid)
                nc.sync.dma_start(adram[:, b0:b0 + CHW], at_t[:])

            # ---- final: y = x * att ----
            acp = per.tile([128, 243], f32, tag="acp")
            nc.sync.dma_start(acp[:],
                              adram.rearrange("a b -> (a b)")
                                   .rearrange("(p n) -> p n", p=128))
            coff = 0
            for ck in CCH:
                xc2 = xp.tile([128, 41 * 64], f32, tag="xa")
                nc.sync.dma_start(xc2[:, :ck * 64],
                                  xin_d[:, coff * 64:(coff + ck) * 64])
                xv2 = xc2[:, :ck * 64].rearrange("p (n c) -> p n c", c=64)
                nc.vector.tensor_tensor(
                    xv2, xv2,
                    acp[:, coff:coff + ck].rearrange("p (n o) -> p n o", o=1)
                       .to_broadcast([128, ck, 64]),
                    op=mybir.AluOpType.mult)
                nc.sync.dma_start(yout_d[:, coff * 64:(coff + ck) * 64],
                                  xc2[:, :ck * 64])
                coff += ck

    nc.compile()
    _BUILD_CACHE[key] = nc
    return nc


def kernel(x_rgb0, feat0, coor0, vox0, feat1, coor1, vox1,
           W_red0, b_red0, W_red2, b_red2, W_red3, b_red3, W_sp, b_sp):
    from concourse.bass_utils import run_bass_kernel_spmd

    a0, a1, lc, cadd, bsp = _fold_weights(
        W_red0, b_red0, W_red2, b_red2, W_red3, b_red3, W_sp, b_sp)
    w3b = np.ascontiguousarray(
        np.broadcast_to(W_red3[0].astype(np.float32), (128, 64)))

    fv0 = np.concatenate([feat0, vox0], axis=-1)   # [B, N0, 35]
    fv1 = np.concatenate([feat1, vox1], axis=-1)   # [B, N1, 67]

    shards = []
    cap0 = cap1 = CAPQ
    for b in range(B):
        for s in range(NSLAB):
            ls0, fvT0, c0 = _bucketize(np.asarray(coor0[b]), fv0[b], s)
            ls1, fvT1, c1 = _bucketize(np.asarray(coor1[b]), fv1[b], s)
            x0 = s * SW
            cl, cr = max(0, x0 - 1), min(W, x0 + SWH - 1)
            xs = np.zeros((H, SWH, 64), np.float32)
            xs[:, cl - (x0 - 1):cr - (x0 - 1), :] = \
                np.asarray(x_rgb0[b, :, :, cl:cr]).transpose(1, 2, 0)
            shards.append(dict(ls0=ls0, fvT0=fvT0, c0=c0,
                               ls1=ls1, fvT1=fvT1, c1=c1,
                               xs=xs, cm=_const_map(s, cadd, bsp)))
            cap0, cap1 = max(cap0, c0), max(cap1, c1)

    nc = _build(cap0, cap1)

    in_maps = []
    for sh in shards:
        fvT0 = np.zeros((35, 128 * cap0), np.float32)
        ls0 = np.full((128, cap0), -1, np.int16)
        c0 = sh["c0"]
        fvT0.reshape(35, 128, cap0)[:, :, :c0] = sh["fvT0"].reshape(35, 128, c0)
        ls0[:, :c0] = sh["ls0"]
        fvT1 = np.zeros((67, 128 * cap1), np.float32)
        ls1 = np.full((128, cap1), -1, np.int16)
        c1 = sh["c1"]
        fvT1.reshape(67, 128, cap1)[:, :, :c1] = sh["fvT1"].reshape(67, 128, c1)
        ls1[:, :c1] = sh["ls1"]
        in_maps.append({
            "xin": np.ascontiguousarray(
                sh["xs"].reshape(NCELL * 64).reshape(128, 243 * 64)),
            "w3b": w3b,
            "fv0": fvT0, "fv1": fvT1,
            "lsx0": ls0, "lsx1": ls1,
            "a0": a0, "a1": a1, "lc": lc, "cm": sh["cm"],
        })

    res = run_bass_kernel_spmd(nc, in_maps, list(range(8)))

    out = np.empty((B, 64, H, W), np.float32)
    k = 0
    for b in range(B):
        for s in range(NSLAB):
            y = res.results[k]["yout"].reshape(NCELL, 64).reshape(H, SWH, 64)
            out[b, :, :, s * SW:(s + 1) * SW] = \
                y[:, 1:1 + SW, :].transpose(2, 0, 1)
            k += 1
    return out
